# revision 12
# baseline (speedup 1.0000x reference)
"""Trainium2 Bass kernel for nn_DiffeomorphicLayer (scaling-and-squaring
diffeomorphic integration):

    flow = velocity / 2**7
    repeat 7x:  flow = flow + trilinear_sample(flow, identity + flow)

Design (v2):
  * The reference's normalize->denormalize round trip cancels algebraically:
    the sample position in voxel coords is exactly v + flow(v).
  * Displacements are small: iterations 0..5 need a [-1..1] per-axis corner
    window, iteration 6 needs [-2..2].  Trilinear sampling becomes an exact
    "spread-weight" stencil with per-axis hat weights
        a(v,t) = relu(1 - |f_a(v) - t|).
  * Sharding: 8 cores = batch (2) x y-quarter (4), fully independent; each
    core computes its 32-row y-slab plus a shrinking halo (8 rows at iter 0
    down to 0), so no collectives.
  * All fields are fp16 (rel tol is 2e-2; fp16 keeps error ~1e-3).  Flow
    ping-pongs through DRAM fp16 slabs [C, 128, 48, 132] (x zero-padded).
  * Compute layout: z on the 128 partitions.  Per 8-row y-superblock, DMA
    stages z-shifted copies of the flow (partition shifts need DMA), scalar
    engine computes hat weights, DVE+GPSIMD compute weight products and
    corner products, and the otherwise-idle TensorEngine accumulates all
    corner terms into PSUM via identity matmuls (exact fp32 accumulation).
    Scalar engine evicts PSUM -> fp16 (fp32 on the last iteration).
"""

import os
import sys
import numpy as np

B, C, D, H, W = 2, 3, 128, 128, 128
NCORES = 8
TIME_STEP = 7

REACH = [1, 1, 1, 1, 1, 1, 2]     # corner window radius per iteration
R = [8, 7, 6, 5, 4, 3, 2, 0]      # y halo rows before iter k
Y_IN = 48                         # y rows staged per core (32 + 2*8)
XP = 2                            # x pad cols per side
WP = W + 2 * XP                   # 132
SBK = 8                           # y rows per superblock
SBH = 4                           # y rows per psum half (N = 4*128 = 512)

NITER = int(os.environ.get("DIFFEO_NITER", str(TIME_STEP)))
GP_FRAC = os.environ.get("DIFFEO_GP", "8/27")     # corner products on gpsimd
ADD_DVE = os.environ.get("DIFFEO_ADD_DVE", "0/27")  # corner adds on DVE

_cache = {}


def _frac(s):
    num, den = s.split("/")
    return int(num), int(den)


def _build_nc():
    try:
        import concourse  # noqa: F401
    except ImportError:
        sys.path.insert(0, "/opt/trn_rl_repo")
    import concourse.bacc as bacc
    import concourse.mybir as mybir
    import concourse.tile as tile

    Op = mybir.AluOpType
    Act = mybir.ActivationFunctionType
    f32 = mybir.dt.float32
    f16 = mybir.dt.float16

    nc = bacc.Bacc("TRN2", target_bir_lowering=False, debug=False,
                   num_devices=NCORES)
    # activation() biases need pre-registered fp32 const APs
    for v in (-2.0, -1.0, 2.0):
        t = nc.alloc_sbuf_tensor(f"const-float32-{v}", [128, 1], f32)
        nc.gpsimd.memset(t.ap(), v)
        nc.const_aps.aps[(f32, v)] = t.ap()
    nc.all_engine_barrier()

    vel = nc.dram_tensor("vel", [C, D, Y_IN, WP], f16, kind="ExternalInput")
    identD = nc.dram_tensor("ident", [128, 128], f16, kind="ExternalInput")
    out = nc.dram_tensor("out", [C, D, 32, W], f32, kind="ExternalOutput")

    gnum, gden = _frac(GP_FRAC)
    anum, aden = _frac(ADD_DVE)

    with tile.TileContext(nc) as tc:
        with (
            tc.tile_pool(name="dram", bufs=1, space="DRAM") as dpool,
            tc.tile_pool(name="sb", bufs=1) as pool,
            tc.tile_pool(name="work", bufs=2) as wpool,
            tc.tile_pool(name="psum", bufs=1, space="PSUM") as ppool,
        ):
            # flow ping-pong, banded in y (16 rows) so cross-iteration
            # dependencies are fine-grained and iterations pipeline
            NBAND = Y_IN // 16
            flow_dram = [[dpool.tile([C, D, 16, WP], f16, tag=f"flow{i}_{bi}",
                                     name=f"flow{i}_{bi}")
                          for bi in range(NBAND)] for i in range(2)]

            def banded_read(bufs, r0, r1):
                """yield (ap, rows_off) pieces of slab rows [r0, r1)."""
                for bi in range(NBAND):
                    s = max(r0, bi * 16)
                    e = min(r1, bi * 16 + 16)
                    if e > s:
                        ap = bufs[bi][:, :, :, :].rearrange(
                            "c z y x -> z c y x")
                        yield ap[:, :, s - bi * 16:e - bi * 16, :], s - r0

            ident = pool.tile([128, 128], f16, tag="ident")
            nc.sync.dma_start(out=ident[:, :], in_=identD.ap())

            # staged z-shifted flow tiles; edge partitions stay zero forever
            fs = {}
            for tz in (-2, -1, 0, 1, 2):
                fs[tz] = pool.tile([128, C, SBK + 4, WP], f16, tag=f"fs{tz}",
                                   name=f"fs{tz}")
                nc.vector.memset(fs[tz][:, :, :, :], 0.0)

            # hat weights, all taps x all axes: [t(5), axis(3), y, x]
            h_all = pool.tile([128, 5, 3, SBK, W], f16, tag="hall")
            # double-buffered copy of the x-axis tap stack; azyx reads this,
            # so h_all itself is free early for the next superblock's hats
            hx_par = [pool.tile([128, 5, SBK, W], f16, tag=f"hx{i}",
                                name=f"hx{i}") for i in range(2)]
            u_t = pool.tile([128, 3, SBK, W], f16, tag="u")

            # eviction staging: fp16 with zeroed x pads (mid iters)
            ev16 = pool.tile([128, C, SBK, WP], f16, tag="ev16")
            nc.vector.memset(ev16[:, :, :, :], 0.0)

            # psum accumulators: (channel, half) -> one 512-col bank
            ps = {(c, h): ppool.tile([128, SBH, W], f32, tag=f"ps{c}{h}",
                                     name=f"ps{c}{h}")
                  for c in range(C) for h in range(2)}

            term_i = [0]
            add_i = [0]
            sb_count = [0]

            for k in range(NITER):
                r = REACH[k]
                S = 2 * r + 1
                lo = 8 - (R[k + 1] if k + 1 < len(R) else 0)
                hi = 40 + (R[k + 1] if k + 1 < len(R) else 0)
                last = (k == NITER - 1)
                src_bufs = None if k == 0 else flow_dram[(k + 1) % 2]
                vel_r = vel.ap().rearrange("c z y x -> z c y x")
                dst_bufs = flow_dram[k % 2]
                outr = out.ap().rearrange("c z y x -> z c y x")

                def stage(dst_tile, psrc, pdst, r0, r1):
                    if src_bufs is None:
                        nc.sync.dma_start(
                            out=dst_tile[pdst, :, :r1 - r0, :],
                            in_=vel_r[psrc, :, r0:r1, :])
                    else:
                        for ap, off in banded_read(src_bufs, r0, r1):
                            n = ap.shape[2]
                            nc.sync.dma_start(
                                out=dst_tile[pdst, :, off:off + n, :],
                                in_=ap[psrc])

                for sb_i, yb in enumerate(range(lo, hi, SBK)):
                    hx = hx_par[sb_count[0] % 2]
                    sb_count[0] += 1
                    ye = min(yb + SBK, hi)
                    yn = ye - yb
                    ym = yn + 2 * r
                    halves = [(0, min(SBH, yn))]
                    if yn > SBH:
                        halves.append((SBH, yn - SBH))

                    # ---- stage z-shifted flow (DMA partition shifts) ----
                    stage(fs[0], slice(None), slice(None), yb - r, ye + r)
                    for tz in range(1, r + 1):
                        stage(fs[tz], slice(tz, None), slice(0, 128 - tz),
                              yb - r, ye + r)
                        stage(fs[-tz], slice(0, 128 - tz), slice(tz, None),
                              yb - r, ye + r)
                    f0 = fs[0]

                    # ---- hat weights on scalar engine ----
                    for t in range(-r, r + 1):
                        nc.scalar.activation(
                            u_t[:, :, :yn, :],
                            f0[:, :, r:r + yn, XP:XP + W],
                            Act.Abs, bias=float(-t))
                        nc.scalar.activation(
                            h_all[:, t + 2, :, :yn, :], u_t[:, :, :yn, :],
                            Act.Relu, bias=1.0, scale=-1.0)
                    nc.scalar.activation(hx[:, 2 - r:3 + r, :yn, :],
                                          h_all[:, 2 - r:3 + r, 2, :yn, :],
                                          Act.Copy)

                    # ---- "+ flow" term starts the psum accumulation ----
                    for c in range(C):
                        for hj, (h0, hn) in enumerate(halves):
                            nc.tensor.matmul(
                                out=ps[(c, hj)][:, :hn, :],
                                lhsT=ident[:, :],
                                rhs=f0[:, c, r + h0:r + h0 + hn, XP:XP + W],
                                start=True, stop=False)

                    # ---- corner products + accumulation ----
                    # tz=0 corners first so fs[0] frees early (lets the next
                    # superblock's staging DMA overlap this one's compute)
                    tz_order = [0, -1, 1, -2, 2][:S]
                    n_corner = 0
                    n_add = 0
                    acc = None
                    for tz in tz_order:
                        azy = wpool.tile([128, 5, SBK, W], f16, tag="azy")
                        nc.vector.tensor_tensor(
                            out=azy[:, :S, :yn, :],
                            in0=h_all[:, tz + 2:tz + 3, 0, :yn, :]
                                .to_broadcast([128, S, yn, W]),
                            in1=h_all[:, 2 - r:3 + r, 1, :yn, :],
                            op=Op.mult)
                        for ty in range(-r, r + 1):
                            iy = ty + r
                            azyx = wpool.tile([128, 5, SBK, W], f16,
                                              tag="azyx")
                            nc.vector.tensor_tensor(
                                out=azyx[:, :S, :yn, :],
                                in0=azy[:, iy:iy + 1, :yn, :]
                                    .to_broadcast([128, S, yn, W]),
                                in1=hx[:, 2 - r:3 + r, :yn, :],
                                op=Op.mult)
                            for tx in range(-r, r + 1):
                                ix = tx + r
                                n_corner += 1
                                is_last = (n_corner == S * S * S)
                                use_gp = (gnum > 0 and
                                          (term_i[0] * gnum) % gden < gnum)
                                term_i[0] += 1
                                use_add = (anum > 0 and
                                           (add_i[0] * anum) % aden < anum)
                                add_i[0] += 1
                                eng = nc.gpsimd if use_gp else nc.vector
                                if use_add and acc is None:
                                    # first DVE-accumulated corner writes the
                                    # accumulator directly
                                    acc = wpool.tile([128, C, SBK, W], f16,
                                                     tag="acc16", bufs=1)
                                    tmp = acc
                                else:
                                    tmp = wpool.tile([128, C, SBK, W], f16,
                                                     tag="tmp_g" if use_gp
                                                     else "tmp_v")
                                eng.tensor_tensor(
                                    out=tmp[:, :, :yn, :],
                                    in0=azyx[:, ix:ix + 1, :yn, :]
                                    .to_broadcast([128, C, yn, W]),
                                    in1=fs[tz][:, :, iy:iy + yn,
                                               XP + tx:XP + tx + W],
                                    op=Op.mult)
                                if use_add:
                                    n_add += 1
                                    if tmp is not acc:
                                        nc.vector.tensor_tensor(
                                            out=acc[:, :, :yn, :],
                                            in0=acc[:, :, :yn, :],
                                            in1=tmp[:, :, :yn, :],
                                            op=Op.add)
                                else:
                                    stop_now = is_last and acc is None
                                    for c in range(C):
                                        for hj, (h0, hn) in enumerate(halves):
                                            nc.tensor.matmul(
                                                out=ps[(c, hj)][:, :hn, :],
                                                lhsT=ident[:, :],
                                                rhs=tmp[:, c, h0:h0 + hn, :],
                                                start=False, stop=stop_now)
                    if acc is not None:
                        # fold the DVE-accumulated partial into psum
                        for c in range(C):
                            for hj, (h0, hn) in enumerate(halves):
                                nc.tensor.matmul(
                                    out=ps[(c, hj)][:, :hn, :],
                                    lhsT=ident[:, :],
                                    rhs=acc[:, c, h0:h0 + hn, :],
                                    start=False, stop=True)

                    # ---- evict psum ----
                    if not last:
                        ev = ev16
                        for c in range(C):
                            for hj, (h0, hn) in enumerate(halves):
                                nc.scalar.activation(
                                    ev[:, c, h0:h0 + hn, XP:XP + W],
                                    ps[(c, hj)][:, :hn, :], Act.Copy)
                        for ap, off in banded_read(dst_bufs, yb, ye):
                            n = ap.shape[2]
                            nc.sync.dma_start(out=ap,
                                              in_=ev[:, :, off:off + n, :])
                    else:
                        for hj, (h0, hn) in enumerate(halves):
                            s_ = max(yb + h0, 8)
                            e_ = min(yb + h0 + hn, 40)
                            if e_ <= s_:
                                continue
                            ev = wpool.tile([128, C, SBH, W], f32,
                                            tag="ev32")
                            for c in range(C):
                                nc.scalar.activation(
                                    ev[:, c, :hn, :],
                                    ps[(c, hj)][:, :hn, :], Act.Copy)
                            o0 = s_ - (yb + h0)
                            nc.sync.dma_start(
                                out=outr[:, :, s_ - 8:e_ - 8, :],
                                in_=ev[:, :, o0:o0 + e_ - s_, :])
    nc.compile()
    return nc


def _get_nc():
    if "nc" not in _cache:
        _cache["nc"] = _build_nc()
    return _cache["nc"]


def run(velocity: np.ndarray, trace: bool = False, **trace_kwargs):
    try:
        import concourse  # noqa: F401
    except ImportError:
        sys.path.insert(0, "/opt/trn_rl_repo")
    from concourse.bass_utils import run_bass_kernel_spmd

    velocity = np.ascontiguousarray(velocity, dtype=np.float32)
    nc = _get_nc()

    scaled = (velocity * np.float32(2.0 ** -TIME_STEP)).astype(np.float16)
    ident = np.eye(128, dtype=np.float16)
    in_maps = []
    for core in range(NCORES):
        b, q = divmod(core, 4)
        slab = np.zeros((C, D, Y_IN, WP), dtype=np.float16)
        y0 = 32 * q - 8
        s0, s1 = max(0, y0), min(H, y0 + Y_IN)
        slab[:, :, s0 - y0:s1 - y0, XP:XP + W] = scaled[b][:, :, s0:s1, :]
        in_maps.append({"vel": slab, "ident": ident})

    res = run_bass_kernel_spmd(nc, in_maps, core_ids=list(range(NCORES)),
                               trace=trace, **trace_kwargs)

    full = np.empty((B, C, D, H, W), dtype=np.float32)
    for core in range(NCORES):
        b, q = divmod(core, 4)
        full[b, :, :, 32 * q:32 * q + 32, :] = res.results[core]["out"]
    return full, res


def kernel(velocity: np.ndarray, sample_grid: np.ndarray) -> np.ndarray:
    """velocity, sample_grid: [2,3,128,128,128] fp32 -> flow [2,3,128,128,128].

    sample_grid is the identity grid by construction; the kernel exploits
    that analytically and does not read it.
    """
    full, _ = run(velocity)
    return full


if __name__ == "__main__":
    v = np.load("/tmp/velocity.npy")
    sg = np.load("/tmp/sample_grid.npy")
    o = kernel(v, sg)
    print("out", o.shape, o.dtype, float(np.abs(o).max()))
